# revision 12
# baseline (speedup 1.0000x reference)
"""BandSplit (LayerNorm + per-band Linear) Trainium2 kernel.

Strategy:
- Data-parallel over batch: 8 batch elements -> 8 NeuronCores (SPMD, no
  collectives).
- Per core: x = spec_ri[b] with shape (1025, 2048, 2) -> out (128, 36, 2048).
- Frames processed in 4 chunks of TC=512.
- Bins packed on SBUF partitions in 9 tiles of <=128 rows; free dim is the
  contiguous (t, c) pair stream, so HBM reads are fully contiguous per bin.
- Per-frame LayerNorm stats (mean, mean-square) for all 36 bands computed with
  indicator matmuls (contract over partitions); rsqrt via sqrt + accurate
  reciprocal; per-band (r, u=mu*r) rows broadcast back to bin-partition layout
  with selector matmuls (step-0 column duplication for the (t,c) interleave);
  normalization on DVE; per-band Linear as fp32r matmuls (TF32-like, 11-bit
  mantissa) with LayerNorm gamma folded into the weights, real/imag split into
  two accumulating matmuls; bias folded into the ScalarE PSUM->SBUF evacuation.
- 16/32-bin bands use PE row-group packing (tile_position) so 4 small-K
  matmuls run concurrently in the 128x128 array.
"""
import numpy as np

BINS = [16] * 20 + [32] * 10 + [64] * 5 + [65]
NB = len(BINS)  # 36
E = 128
B = 8
T = 2048
TC = 512
NCHUNK = T // TC
EPS = 1e-5

# Tile layout: bins packed densely, but band 34 (64 bins) and band 35 (65
# bins) get their own tiles so no band straddles a 128-partition tile.
# tiles 0..6: bins 0..895 (128 each), tile 7: bins 896..959 (band 34),
# tile 8: bins 960..1024 (band 35).
TILE_BIN_START = [0, 128, 256, 384, 512, 640, 768, 896, 960]
TILE_BIN_COUNT = [128, 128, 128, 128, 128, 128, 128, 64, 65]
NTILES = 9

BAND_START = np.cumsum([0] + BINS)[:-1]  # bin index where each band starts


def _band_layout():
    """Per band: (tile_j, local_row0, nbins, kslot_base, kslot_size, parity)."""
    layout = []
    for beta, nb in enumerate(BINS):
        b0 = int(BAND_START[beta])
        for j in range(NTILES):
            if TILE_BIN_START[j] <= b0 < TILE_BIN_START[j] + TILE_BIN_COUNT[j]:
                break
        r0 = b0 - TILE_BIN_START[j]
        assert r0 + nb <= TILE_BIN_COUNT[j], (beta, j, r0, nb)
        if nb == 16:
            kbase = (r0 // 32) * 32
            ksize = 32
            parity = (r0 // 16) % 2
        elif nb == 32:
            assert r0 % 32 == 0
            kbase, ksize, parity = r0, 32, 0
        elif nb == 64:
            assert r0 % 64 == 0
            kbase, ksize, parity = r0, 64, 0
        else:  # 65
            assert r0 == 0
            kbase, ksize, parity = 0, 65, 0
        layout.append((j, r0, nb, kbase, ksize, parity))
    return layout


BAND_LAYOUT = _band_layout()

TILE_BANDS = [[] for _ in range(NTILES)]  # per tile: list of band indices
for _beta, (_j, *_rest) in enumerate(BAND_LAYOUT):
    TILE_BANDS[_j].append(_beta)

_CACHE = {}


def _build(passes=1, bench=False):
    """Build + schedule the Bass module once. Returns (nc, names)."""
    key = ("nc", passes, bench)
    if key in _CACHE:
        return _CACHE[key]

    from contextlib import ExitStack
    import concourse.tile as tile
    from concourse import bacc, mybir

    F32 = mybir.dt.float32
    F32R = mybir.dt.float32r
    AF = mybir.ActivationFunctionType
    ALU = mybir.AluOpType

    nc = bacc.Bacc("TRN2", target_bir_lowering=False, debug=False)

    x_d = nc.dram_tensor("x", [1025, T, 2], F32R, kind="ExternalInput").ap()
    # weights packed per (c, parity): rows = tile_j*128 + local_row
    w_d = nc.dram_tensor("w", [2, 2, NTILES * 128, E], F32R,
                         kind="ExternalInput").ap()
    ind_d = nc.dram_tensor("ind", [NTILES, 128, 2 * NB], F32R,
                           kind="ExternalInput").ap()  # [:, :, :36]=mu  [36:]=sq
    sel_d = nc.dram_tensor("sel", [NTILES, NB, 128], F32R,
                           kind="ExternalInput").ap()
    bias_d = nc.dram_tensor("bias", [E, NB], F32, kind="ExternalInput").ap()
    invd_d = nc.dram_tensor("invd", [NB, 1], F32, kind="ExternalInput").ap()
    eps_d = nc.dram_tensor("eps", [NB, 1], F32, kind="ExternalInput").ap()
    if bench:
        outx_d = nc.dram_tensor("out", [E, 1], F32, kind="ExternalOutput").ap()
        out_d = None
    else:
        out_d = nc.dram_tensor("out", [E, NB, T], F32,
                               kind="ExternalOutput").ap()

    with tile.TileContext(nc) as tc, ExitStack() as ctx:
        consts = ctx.enter_context(tc.tile_pool(name="consts", bufs=1))
        xpool = ctx.enter_context(tc.tile_pool(name="x", bufs=2))
        x2pool = ctx.enter_context(tc.tile_pool(name="x2", bufs=3))
        xspool = ctx.enter_context(tc.tile_pool(name="xs", bufs=4))
        stpool = ctx.enter_context(tc.tile_pool(name="stats", bufs=2))
        outpool = ctx.enter_context(tc.tile_pool(name="out", bufs=6))
        ps_st = ctx.enter_context(tc.tile_pool(name="ps_st", bufs=1, space="PSUM"))
        ps_sel = ctx.enter_context(tc.tile_pool(name="ps_sel", bufs=1, space="PSUM"))
        ps_z = ctx.enter_context(tc.tile_pool(name="ps_z", bufs=2, space="PSUM"))
        if bench:
            drampool = ctx.enter_context(tc.tile_pool(name="dscr", bufs=1, space="DRAM"))
            out_d = drampool.tile([E, NB, T], F32, name="out_scratch")

        # constants
        w_s = [[consts.tile([128, E], F32R, tag=f"w{c}{p}{j}", name=f"w{c}{p}{j}")
                for j in range(NTILES)] for c in range(2) for p in range(2)]
        # index helper: w_s[c*2+p][j]
        for c in range(2):
            for p in range(2):
                for j in range(NTILES):
                    if p == 1 and j > 2:
                        continue  # parity-1 weights only exist for 16-bin tiles
                    nc.sync.dma_start(
                        w_s[c * 2 + p][j][:],
                        w_d[c, p, j * 128:(j + 1) * 128, :])
        ind_s = [consts.tile([128, 2 * NB], F32R, tag=f"ind{j}", name=f"ind{j}")
                 for j in range(NTILES)]
        sel_s = [consts.tile([NB, 128], F32R, tag=f"sel{j}", name=f"sel{j}")
                 for j in range(NTILES)]
        for j in range(NTILES):
            nc.sync.dma_start(ind_s[j][:], ind_d[j])
            nc.sync.dma_start(sel_s[j][:], sel_d[j])
        bias_s = consts.tile([E, NB], F32)
        nc.sync.dma_start(bias_s[:], bias_d[:])
        invd_s = consts.tile([NB, 1], F32)
        nc.sync.dma_start(invd_s[:], invd_d[:])
        eps_s = consts.tile([NB, 1], F32)
        nc.sync.dma_start(eps_s[:], eps_d[:])

        def stage_stats(k):
            """Load chunk k, square, stats matmuls, produce r_sb/u_sb."""
            t0 = (k % NCHUNK) * TC
            xts = []
            for j in range(NTILES):
                nb = TILE_BIN_COUNT[j]
                xt = xpool.tile([128, 2 * TC], F32R, tag=f"x{j}", name=f"x_{k}_{j}")
                src = x_d[TILE_BIN_START[j]:TILE_BIN_START[j] + nb,
                          t0:t0 + TC, :].rearrange("p t c -> p (t c)")
                nc.sync.dma_start(xt[0:nb, :], src)
                xts.append(xt)

            # squares (ScalarE, keeps DVE free)
            x2s = []
            for j in range(NTILES):
                nb = TILE_BIN_COUNT[j]
                x2 = x2pool.tile([128, 2 * TC], F32R, tag="x2", name=f"x2_{k}_{j}")
                nc.scalar.activation(x2[0:nb, :], xts[j][0:nb, :], AF.Square)
                x2s.append(x2)

            # stats matmuls: mu_ps/sq_ps [36, TC] accumulated over tiles+c
            mu_ps = ps_st.tile([NB, TC], F32, tag="mu", name=f"mu_{k}")
            sq_ps = ps_st.tile([NB, TC], F32, tag="sq", name=f"sq_{k}")
            n_acc = NTILES * 2
            i = 0
            for j in range(NTILES):
                nb = TILE_BIN_COUNT[j]
                for c in range(2):
                    xv = xts[j][0:nb, :].rearrange("p (t c) -> p t c", c=2)
                    x2v = x2s[j][0:nb, :].rearrange("p (t c) -> p t c", c=2)
                    nc.tensor.matmul(mu_ps[:], ind_s[j][0:nb, 0:NB],
                                     xv[:, :, c], start=(i == 0),
                                     stop=(i == n_acc - 1), skip_group_check=True)
                    nc.tensor.matmul(sq_ps[:], ind_s[j][0:nb, NB:2 * NB],
                                     x2v[:, :, c], start=(i == 0),
                                     stop=(i == n_acc - 1), skip_group_check=True)
                    i += 1

            # stats post: r = 1/sqrt(var+eps), u = mu*r  (rows [36, TC])
            mu_sb = stpool.tile([NB, TC], F32, tag="mu_sb", name=f"mu_sb_{k}")
            nc.vector.tensor_scalar(mu_sb[:], mu_ps[:], invd_s[:, 0:1], None,
                                    ALU.mult)
            mu2 = stpool.tile([NB, TC], F32, tag="mu2", name=f"mu2_{k}")
            nc.vector.tensor_mul(mu2[:], mu_sb[:], mu_sb[:])
            var = stpool.tile([NB, TC], F32, tag="var", name=f"var_{k}")
            # var = sq * invd - mu^2
            nc.vector.scalar_tensor_tensor(var[:], sq_ps[:], invd_s[:, 0:1],
                                           mu2[:], ALU.mult, ALU.subtract)
            sd = stpool.tile([NB, TC], F32, tag="sd", name=f"sd_{k}")
            nc.scalar.activation(sd[:], var[:], AF.Sqrt, bias=eps_s[:, 0:1])
            rr = stpool.tile([NB, TC], F32, tag="rr", name=f"rr_{k}")
            scr = stpool.tile([NB, TC], F32, tag="scr", name=f"scr_{k}")
            nc.vector.reciprocal_approx_accurate(rr[:], sd[:], scr[:])
            r_sb = stpool.tile([NB, TC], F32R, tag="r_sb", name=f"r_sb_{k}")
            nc.vector.tensor_copy(r_sb[:], rr[:])
            u_sb = stpool.tile([NB, TC], F32R, tag="u_sb", name=f"u_sb_{k}")
            nc.vector.tensor_mul(u_sb[:], mu_sb[:], rr[:])
            return xts, r_sb, u_sb

        def stage_z(k, xts, r_sb, u_sb):
            """Selects, normalize, per-band matmuls, evac, store for chunk k."""
            t0 = (k % NCHUNK) * TC
            for j in range(NTILES):
                nb = TILE_BIN_COUNT[j]
                r2d = ps_sel.tile([128, 2 * TC], F32, tag="r2d", name=f"r2d_{k}_{j}")
                m2d = ps_sel.tile([128, 2 * TC], F32, tag="m2d", name=f"m2d_{k}_{j}")
                H = TC // 2
                for h in range(2):
                    rv = r_sb[:, h * H:(h + 1) * H].to_broadcast((NB, H, 2))
                    uv = u_sb[:, h * H:(h + 1) * H].to_broadcast((NB, H, 2))
                    nc.tensor.matmul(
                        r2d[0:nb, 2 * h * H:2 * (h + 1) * H]
                        .rearrange("p (t c) -> p t c", c=2),
                        sel_s[j][:, 0:nb], rv, start=True, stop=True)
                    nc.tensor.matmul(
                        m2d[0:nb, 2 * h * H:2 * (h + 1) * H]
                        .rearrange("p (t c) -> p t c", c=2),
                        sel_s[j][:, 0:nb], uv, start=True, stop=True)
                # xs = x * r2d - m2d   (x * r_band - mu_band*r_band)
                xs = xspool.tile([128, 2 * TC], F32R, tag="xs", name=f"xs_{k}_{j}")
                nc.vector.tensor_mul(xs[0:nb, :], xts[j][0:nb, :].bitcast(F32),
                                     r2d[0:nb, :])
                nc.vector.tensor_sub(xs[0:nb, :], xs[0:nb, :].bitcast(F32),
                                     m2d[0:nb, :])
                xsr = xs[:]

                # ---- per-band matmuls + evacuation
                for beta in TILE_BANDS[j]:
                    _, r0, nbb, kbase, ksize, par = BAND_LAYOUT[beta]
                    zps = ps_z.tile([E, TC], F32, tag="z", name=f"z_{k}_{beta}")
                    kslice = slice(kbase, kbase + ksize)
                    tp = (kbase % 128, 0) if ksize <= 64 else (0, 0)
                    xsv = xsr[kslice, :].rearrange("p (t c) -> p t c", c=2)
                    for c in range(2):
                        nc.tensor.matmul(
                            zps[:], w_s[c * 2 + par][j][kslice, :],
                            xsv[:, :, c], start=(c == 0), stop=(c == 1),
                            tile_position=tp, skip_group_check=True)
                    y_sb = outpool.tile([E, TC], F32, tag="y", bufs=6,
                                        name=f"y_{k}_{beta}")
                    nc.scalar.activation(y_sb[:], zps[:], AF.Identity,
                                         bias=bias_s[:, beta:beta + 1])
                    nc.sync.dma_start(out_d[:, beta, t0:t0 + TC], y_sb[:])

        # software pipeline: chunk k+1's load/stats run while chunk k's
        # select/normalize/matmul phase executes (keeps PE dense across the
        # serial stats-post chain).
        if bench:
            dummy = consts.tile([E, 1], F32)
            nc.vector.tensor_copy(dummy[:], bias_s[:, 0:1])
            nc.sync.dma_start(outx_d[:], dummy[:])
        nk = NCHUNK * passes
        pending = stage_stats(0)
        for k in range(nk):
            nxt = stage_stats(k + 1) if k + 1 < nk else None
            stage_z(k, *pending)
            pending = nxt

    nc.compile()
    _CACHE[key] = nc
    return nc


def _round_f32r(a):
    """Round fp32 array to fp32r grid (11-bit mantissa, round-to-nearest)."""
    u = a.astype(np.float32).view(np.uint32)
    u = (u + 0x800 + ((u >> 12) & 1)).astype(np.uint32) & np.uint32(0xFFFFF000)
    return u.view(np.float32)


def _host_constants(norm_w, norm_b, lin_w, lin_b):
    """Fold LN gamma/beta into the linear weights, pack to tile layout."""
    w_np = np.zeros((2, 2, NTILES * 128, E), dtype=np.float32)
    bias_np = np.zeros((E, NB), dtype=np.float32)
    ind_np = np.zeros((NTILES, 128, 2 * NB), dtype=np.float32)
    sel_np = np.zeros((NTILES, NB, 128), dtype=np.float32)
    invd_np = np.zeros((NB, 1), dtype=np.float32)

    for beta, nb in enumerate(BINS):
        j, r0, _, _, _, par = BAND_LAYOUT[beta]
        d = 2 * nb
        Wg = np.asarray(lin_w[beta], dtype=np.float64) * \
            np.asarray(norm_w[beta], dtype=np.float64)[None, :]      # [E, d]
        bp = np.asarray(lin_b[beta], dtype=np.float64) + \
            np.asarray(lin_w[beta], dtype=np.float64) @ \
            np.asarray(norm_b[beta], dtype=np.float64)               # [E]
        # column 2k+c of Wg -> w[c, par, tile_row(r0+k), :]
        for c in range(2):
            w_np[c, par, j * 128 + r0: j * 128 + r0 + nb, :] = \
                Wg[:, c::2].T.astype(np.float32)
        bias_np[:, beta] = bp.astype(np.float32)
        ind_np[j, r0:r0 + nb, beta] = 1.0
        ind_np[j, r0:r0 + nb, NB + beta] = 1.0
        sel_np[j, beta, r0:r0 + nb] = 1.0
        invd_np[beta, 0] = 1.0 / d

    w_np = _round_f32r(w_np)
    return w_np, bias_np, ind_np, sel_np, invd_np


def kernel(spec_ri, norm_w, norm_b, lin_w, lin_b):
    from concourse import bass_utils

    spec = np.asarray(spec_ri, dtype=np.float32)
    assert spec.shape == (B, 1025, T, 2), spec.shape

    nc = _build()
    w_np, bias_np, ind_np, sel_np, invd_np = _host_constants(
        norm_w, norm_b, lin_w, lin_b)

    in_maps = []
    for b in range(B):
        in_maps.append({
            "x": np.ascontiguousarray(spec[b]),
            "w": w_np, "ind": ind_np, "sel": sel_np,
            "bias": bias_np, "invd": invd_np,
            "eps": np.full((NB, 1), EPS, dtype=np.float32),
        })
    res = bass_utils.run_bass_kernel_spmd(nc, in_maps, core_ids=list(range(B)))
    out = np.stack([res.results[b]["out"] for b in range(B)], axis=0)
    return out


# revision 16
# speedup vs baseline: 46.3958x; 46.3958x over previous
"""BandSplit (LayerNorm + per-band Linear) Trainium2 kernel.

Strategy:
- Data-parallel over batch: 8 batch elements -> 8 NeuronCores (SPMD, no
  collectives).
- Per core: x = spec_ri[b] with shape (1025, 2048, 2) -> out (128, 36, 2048).
- Frames processed in 4 chunks of TC=512.
- Bins packed on SBUF partitions in 9 tiles of <=128 rows; free dim is the
  contiguous (t, c) pair stream, so HBM reads are fully contiguous per bin.
- Per-frame LayerNorm stats (mean, mean-square) for all 36 bands computed with
  indicator matmuls (contract over partitions); rsqrt via sqrt + accurate
  reciprocal; per-band (r, u=mu*r) rows broadcast back to bin-partition layout
  with selector matmuls (step-0 column duplication for the (t,c) interleave);
  normalization on DVE; per-band Linear as fp32r matmuls (TF32-like, 11-bit
  mantissa) with LayerNorm gamma folded into the weights, real/imag split into
  two accumulating matmuls; bias folded into the ScalarE PSUM->SBUF evacuation.
- 16/32-bin bands use PE row-group packing (tile_position) so 4 small-K
  matmuls run concurrently in the 128x128 array.
"""
import numpy as np

BINS = [16] * 20 + [32] * 10 + [64] * 5 + [65]
NB = len(BINS)  # 36
E = 128
B = 8
T = 2048
TC = 512
NCHUNK = T // TC
EPS = 1e-5

# Tile layout: bins packed densely, but band 34 (64 bins) and band 35 (65
# bins) get their own tiles so no band straddles a 128-partition tile.
# tiles 0..6: bins 0..895 (128 each), tile 7: bins 896..959 (band 34),
# tile 8: bins 960..1024 (band 35).
TILE_BIN_START = [0, 128, 256, 384, 512, 640, 768, 896, 960]
TILE_BIN_COUNT = [128, 128, 128, 128, 128, 128, 128, 64, 65]
NTILES = 9

BAND_START = np.cumsum([0] + BINS)[:-1]  # bin index where each band starts


def _band_layout():
    """Per band: (tile_j, local_row0, nbins, kslot_base, kslot_size, parity)."""
    layout = []
    for beta, nb in enumerate(BINS):
        b0 = int(BAND_START[beta])
        for j in range(NTILES):
            if TILE_BIN_START[j] <= b0 < TILE_BIN_START[j] + TILE_BIN_COUNT[j]:
                break
        r0 = b0 - TILE_BIN_START[j]
        assert r0 + nb <= TILE_BIN_COUNT[j], (beta, j, r0, nb)
        if nb == 16:
            kbase = (r0 // 32) * 32
            ksize = 32
            parity = (r0 // 16) % 2
        elif nb == 32:
            assert r0 % 32 == 0
            kbase, ksize, parity = r0, 32, 0
        elif nb == 64:
            assert r0 % 64 == 0
            kbase, ksize, parity = r0, 64, 0
        else:  # 65
            assert r0 == 0
            kbase, ksize, parity = 0, 65, 0
        layout.append((j, r0, nb, kbase, ksize, parity))
    return layout


BAND_LAYOUT = _band_layout()

TILE_BANDS = [[] for _ in range(NTILES)]  # per tile: list of band indices
for _beta, (_j, *_rest) in enumerate(BAND_LAYOUT):
    TILE_BANDS[_j].append(_beta)
# Emission order: round-robin across PE row-group slots so consecutive
# self-loading matmuls target different 32-row strips and overlap.
for _j in range(NTILES):
    _bs = TILE_BANDS[_j]
    _by_slot = {}
    for _b in _bs:
        _by_slot.setdefault(BAND_LAYOUT[_b][3], []).append(_b)
    _order = []
    _slots = sorted(_by_slot)
    _i = 0
    while any(_by_slot.values()):
        _s = _slots[_i % len(_slots)]
        if _by_slot[_s]:
            _order.append(_by_slot[_s].pop(0))
        _i += 1
    TILE_BANDS[_j] = _order

_CACHE = {}


def _build(passes=1, bench=False, ablate=None):
    """Build + schedule the Bass module once. Returns (nc, names)."""
    key = ("nc", passes, bench, ablate)
    if key in _CACHE:
        return _CACHE[key]

    from contextlib import ExitStack
    import concourse.tile as tile
    from concourse import bacc, mybir

    F32 = mybir.dt.float32
    F32R = mybir.dt.float32r
    AF = mybir.ActivationFunctionType
    ALU = mybir.AluOpType

    nc = bacc.Bacc("TRN2", target_bir_lowering=False, debug=False)

    x_d = nc.dram_tensor("x", [1025, T, 2], F32R, kind="ExternalInput").ap()
    # weights packed per (c, parity): rows = tile_j*128 + local_row
    w_d = nc.dram_tensor("w", [2, 2, NTILES * 128, E], F32R,
                         kind="ExternalInput").ap()
    ind_d = nc.dram_tensor("ind", [NTILES, 128, 2 * NB], F32R,
                           kind="ExternalInput").ap()  # [:, :, :36]=mu  [36:]=sq
    sel_d = nc.dram_tensor("sel", [NTILES, NB, 128], F32R,
                           kind="ExternalInput").ap()
    bias_d = nc.dram_tensor("bias", [E, NB], F32, kind="ExternalInput").ap()
    invd_d = nc.dram_tensor("invd", [NB, 1], F32, kind="ExternalInput").ap()
    eps_d = nc.dram_tensor("eps", [NB, 1], F32, kind="ExternalInput").ap()
    if bench:
        outx_d = nc.dram_tensor("out", [E, 1], F32, kind="ExternalOutput").ap()
        out_d = None
    else:
        out_d = nc.dram_tensor("out", [E, NB, T], F32,
                               kind="ExternalOutput").ap()

    with tile.TileContext(nc) as tc, ExitStack() as ctx:
        consts = ctx.enter_context(tc.tile_pool(name="consts", bufs=1))
        xpool = ctx.enter_context(tc.tile_pool(name="x", bufs=2))
        x2pool = ctx.enter_context(tc.tile_pool(name="x2", bufs=3))
        xspool = ctx.enter_context(tc.tile_pool(name="xs", bufs=4))
        stpool = ctx.enter_context(tc.tile_pool(name="stats", bufs=2))
        outpool = ctx.enter_context(tc.tile_pool(name="out", bufs=6))
        ps_st = ctx.enter_context(tc.tile_pool(name="ps_st", bufs=1, space="PSUM"))
        ps_sel = ctx.enter_context(tc.tile_pool(name="ps_sel", bufs=1, space="PSUM"))
        ps_z = ctx.enter_context(tc.tile_pool(name="ps_z", bufs=2, space="PSUM"))
        if bench:
            drampool = ctx.enter_context(tc.tile_pool(name="dscr", bufs=1, space="DRAM"))
            out_d = drampool.tile([E, NB, T], F32, name="out_scratch")

        # constants
        w_s = [[consts.tile([128, E], F32R, tag=f"w{c}{p}{j}", name=f"w{c}{p}{j}")
                for j in range(NTILES)] for c in range(2) for p in range(2)]
        # index helper: w_s[c*2+p][j]
        for c in range(2):
            for p in range(2):
                for j in range(NTILES):
                    if p == 1 and j > 2:
                        continue  # parity-1 weights only exist for 16-bin tiles
                    nc.sync.dma_start(
                        w_s[c * 2 + p][j][:],
                        w_d[c, p, j * 128:(j + 1) * 128, :])
        ind_s = [consts.tile([128, 2 * NB], F32R, tag=f"ind{j}", name=f"ind{j}")
                 for j in range(NTILES)]
        sel_s = [consts.tile([NB, 128], F32R, tag=f"sel{j}", name=f"sel{j}")
                 for j in range(NTILES)]
        for j in range(NTILES):
            nc.sync.dma_start(ind_s[j][:], ind_d[j])
            nc.sync.dma_start(sel_s[j][:], sel_d[j])
        bias_s = consts.tile([E, NB], F32)
        nc.sync.dma_start(bias_s[:], bias_d[:])
        invd_s = consts.tile([NB, 1], F32)
        nc.sync.dma_start(invd_s[:], invd_d[:])
        eps_s = consts.tile([NB, 1], F32)
        nc.sync.dma_start(eps_s[:], eps_d[:])

        def stage_stats(k):
            """Load chunk k, square, stats matmuls, produce r_sb/u_sb."""
            t0 = (k % NCHUNK) * TC
            xts = []
            for j in range(NTILES):
                nb = TILE_BIN_COUNT[j]
                xt = xpool.tile([128, 2 * TC], F32R, tag=f"x{j}", name=f"x_{k}_{j}")
                src = x_d[TILE_BIN_START[j]:TILE_BIN_START[j] + nb,
                          t0:t0 + TC, :].rearrange("p t c -> p (t c)")
                nc.sync.dma_start(xt[0:nb, :], src)
                xts.append(xt)

            # squares on DVE (keeps ScalarE on a single activation table)
            x2s = []
            for j in range(NTILES):
                nb = TILE_BIN_COUNT[j]
                x2 = x2pool.tile([128, 2 * TC], F32R, tag="x2", name=f"x2_{k}_{j}")
                nc.vector.tensor_mul(x2[0:nb, :], xts[j][0:nb, :].bitcast(F32),
                                     xts[j][0:nb, :].bitcast(F32))
                x2s.append(x2)

            # stats matmuls: mu_ps/sq_ps [36, TC] accumulated over tiles+c
            if ablate == "dma":
                return None, None, None
            mu_ps = ps_st.tile([NB, TC], F32, tag="mu", name=f"mu_{k}")
            sq_ps = ps_st.tile([NB, TC], F32, tag="sq", name=f"sq_{k}")
            n_acc = NTILES * 2
            i = 0
            for j in range(NTILES):
                nb = TILE_BIN_COUNT[j]
                for c in range(2):
                    xv = xts[j][0:nb, :].rearrange("p (t c) -> p t c", c=2)
                    x2v = x2s[j][0:nb, :].rearrange("p (t c) -> p t c", c=2)
                    nc.tensor.matmul(mu_ps[:], ind_s[j][0:nb, 0:NB],
                                     xv[:, :, c], start=(i == 0),
                                     stop=(i == n_acc - 1), skip_group_check=True)
                    nc.tensor.matmul(sq_ps[:], ind_s[j][0:nb, NB:2 * NB],
                                     x2v[:, :, c], start=(i == 0),
                                     stop=(i == n_acc - 1), skip_group_check=True)
                    i += 1

            # stats post: r = 1/sqrt(var+eps), u = mu*r  (rows [36, TC])
            mu_sb = stpool.tile([NB, TC], F32, tag="mu_sb", name=f"mu_sb_{k}")
            nc.vector.tensor_scalar(mu_sb[:], mu_ps[:], invd_s[:, 0:1], None,
                                    ALU.mult)
            mu2 = stpool.tile([NB, TC], F32, tag="mu2", name=f"mu2_{k}")
            nc.vector.tensor_mul(mu2[:], mu_sb[:], mu_sb[:])
            var = stpool.tile([NB, TC], F32, tag="var", name=f"var_{k}")
            # var = sq * invd - mu^2
            nc.vector.scalar_tensor_tensor(var[:], sq_ps[:], invd_s[:, 0:1],
                                           mu2[:], ALU.mult, ALU.subtract)
            sd = stpool.tile([NB, TC], F32, tag="sd", name=f"sd_{k}")
            nc.scalar.activation(sd[:], var[:], AF.Sqrt, bias=eps_s[:, 0:1])
            rr = stpool.tile([NB, TC], F32, tag="rr", name=f"rr_{k}")
            scr = stpool.tile([NB, TC], F32, tag="scr", name=f"scr_{k}")
            nc.vector.reciprocal_approx_accurate(rr[:], sd[:], scr[:])
            r_sb = stpool.tile([NB, TC], F32R, tag="r_sb", name=f"r_sb_{k}")
            nc.vector.tensor_copy(r_sb[:], rr[:])
            u_sb = stpool.tile([NB, TC], F32R, tag="u_sb", name=f"u_sb_{k}")
            nc.vector.tensor_mul(u_sb[:], mu_sb[:], rr[:])
            return xts, r_sb, u_sb

        def stage_z(k, xts, r_sb, u_sb):
            """Selects, normalize, per-band matmuls, evac, store for chunk k."""
            if ablate == "dma":
                return
            t0 = (k % NCHUNK) * TC
            for j in range(NTILES):
                nb = TILE_BIN_COUNT[j]
                r2d = ps_sel.tile([128, 2 * TC], F32, tag="r2d", name=f"r2d_{k}_{j}")
                m2d = ps_sel.tile([128, 2 * TC], F32, tag="m2d", name=f"m2d_{k}_{j}")
                H = TC // 2
                for h in range(2):
                    rv = r_sb[:, h * H:(h + 1) * H].to_broadcast((NB, H, 2))
                    uv = u_sb[:, h * H:(h + 1) * H].to_broadcast((NB, H, 2))
                    nc.tensor.matmul(
                        r2d[0:nb, 2 * h * H:2 * (h + 1) * H]
                        .rearrange("p (t c) -> p t c", c=2),
                        sel_s[j][:, 0:nb], rv, start=True, stop=True)
                    nc.tensor.matmul(
                        m2d[0:nb, 2 * h * H:2 * (h + 1) * H]
                        .rearrange("p (t c) -> p t c", c=2),
                        sel_s[j][:, 0:nb], uv, start=True, stop=True)
                # xs = x * r2d - m2d   (x * r_band - mu_band*r_band)
                xs = xspool.tile([128, 2 * TC], F32R, tag="xs", name=f"xs_{k}_{j}")
                nc.vector.tensor_mul(xs[0:nb, :], xts[j][0:nb, :].bitcast(F32),
                                     r2d[0:nb, :])
                nc.vector.tensor_sub(xs[0:nb, :], xs[0:nb, :].bitcast(F32),
                                     m2d[0:nb, :])
                xsr = xs[:]

                # ---- per-band matmuls + evacuation
                for beta in (TILE_BANDS[j] if ablate != "noz" else []):
                    _, r0, nbb, kbase, ksize, par = BAND_LAYOUT[beta]
                    zps = ps_z.tile([E, TC], F32, tag="z", name=f"z_{k}_{beta}")
                    kslice = slice(kbase, kbase + ksize)
                    tp = (kbase % 128, 0) if ksize <= 64 else (0, 0)
                    xsv = xsr[kslice, :].rearrange("p (t c) -> p t c", c=2)
                    for c in range(2):
                        nc.tensor.matmul(
                            zps[:], w_s[c * 2 + par][j][kslice, :],
                            xsv[:, :, c], start=(c == 0), stop=(c == 1),
                            tile_position=tp, skip_group_check=True)
                    y_sb = outpool.tile([E, TC], F32, tag="y", bufs=6,
                                        name=f"y_{k}_{beta}")
                    nc.scalar.activation(y_sb[:], zps[:], AF.Identity,
                                         bias=bias_s[:, beta:beta + 1])
                    nc.sync.dma_start(out_d[:, beta, t0:t0 + TC], y_sb[:])

        # software pipeline: chunk k+1's load/stats run while chunk k's
        # select/normalize/matmul phase executes (keeps PE dense across the
        # serial stats-post chain).
        if bench:
            dummy = consts.tile([E, 1], F32)
            nc.vector.tensor_copy(dummy[:], bias_s[:, 0:1])
            nc.sync.dma_start(outx_d[:], dummy[:])
        nk = NCHUNK * passes
        pending = stage_stats(0)
        for k in range(nk):
            nxt = stage_stats(k + 1) if k + 1 < nk else None
            if pending is not None and pending[0] is not None:
                stage_z(k, *pending)
            pending = nxt

    nc.compile()
    _CACHE[key] = nc
    return nc


def _round_f32r(a):
    """Round fp32 array to fp32r grid (11-bit mantissa, round-to-nearest)."""
    u = a.astype(np.float32).view(np.uint32)
    u = (u + 0x800 + ((u >> 12) & 1)).astype(np.uint32) & np.uint32(0xFFFFF000)
    return u.view(np.float32)


def _host_constants(norm_w, norm_b, lin_w, lin_b):
    """Fold LN gamma/beta into the linear weights, pack to tile layout."""
    w_np = np.zeros((2, 2, NTILES * 128, E), dtype=np.float32)
    bias_np = np.zeros((E, NB), dtype=np.float32)
    ind_np = np.zeros((NTILES, 128, 2 * NB), dtype=np.float32)
    sel_np = np.zeros((NTILES, NB, 128), dtype=np.float32)
    invd_np = np.zeros((NB, 1), dtype=np.float32)

    for beta, nb in enumerate(BINS):
        j, r0, _, _, _, par = BAND_LAYOUT[beta]
        d = 2 * nb
        Wg = np.asarray(lin_w[beta], dtype=np.float64) * \
            np.asarray(norm_w[beta], dtype=np.float64)[None, :]      # [E, d]
        bp = np.asarray(lin_b[beta], dtype=np.float64) + \
            np.asarray(lin_w[beta], dtype=np.float64) @ \
            np.asarray(norm_b[beta], dtype=np.float64)               # [E]
        # column 2k+c of Wg -> w[c, par, tile_row(r0+k), :]
        for c in range(2):
            w_np[c, par, j * 128 + r0: j * 128 + r0 + nb, :] = \
                Wg[:, c::2].T.astype(np.float32)
        bias_np[:, beta] = bp.astype(np.float32)
        ind_np[j, r0:r0 + nb, beta] = 1.0
        ind_np[j, r0:r0 + nb, NB + beta] = 1.0
        sel_np[j, beta, r0:r0 + nb] = 1.0
        invd_np[beta, 0] = 1.0 / d

    w_np = _round_f32r(w_np)
    return w_np, bias_np, ind_np, sel_np, invd_np


def kernel(spec_ri, norm_w, norm_b, lin_w, lin_b):
    from concourse import bass_utils

    spec = _round_f32r(np.asarray(spec_ri, dtype=np.float32))
    assert spec.shape == (B, 1025, T, 2), spec.shape

    nc = _build()
    w_np, bias_np, ind_np, sel_np, invd_np = _host_constants(
        norm_w, norm_b, lin_w, lin_b)

    in_maps = []
    for b in range(B):
        in_maps.append({
            "x": np.ascontiguousarray(spec[b]),
            "w": w_np, "ind": ind_np, "sel": sel_np,
            "bias": bias_np, "invd": invd_np,
            "eps": np.full((NB, 1), EPS, dtype=np.float32),
        })
    res = bass_utils.run_bass_kernel_spmd(nc, in_maps, core_ids=list(range(B)))
    out = np.stack([res.results[b]["out"] for b in range(B)], axis=0)
    return out


# revision 25
# speedup vs baseline: 47.5804x; 1.0255x over previous
"""BandSplit (LayerNorm + per-band Linear) Trainium2 kernel.

Strategy:
- Data-parallel over batch: 8 batch elements -> 8 NeuronCores (SPMD, no
  collectives).
- Per core: x = spec_ri[b] with shape (1025, 2048, 2) -> out (128, 36, 2048).
- Frames processed in 4 chunks of TC=512.
- Bins packed on SBUF partitions in 9 tiles of <=128 rows; free dim is the
  contiguous (t, c) pair stream, so HBM reads are fully contiguous per bin.
- Per-frame LayerNorm stats (mean, mean-square) for all 36 bands computed with
  indicator matmuls (contract over partitions); rsqrt via sqrt + accurate
  reciprocal; per-band (r, u=mu*r) rows broadcast back to bin-partition layout
  with selector matmuls (step-0 column duplication for the (t,c) interleave);
  normalization on DVE; per-band Linear as fp32r matmuls (TF32-like, 11-bit
  mantissa) with LayerNorm gamma folded into the weights, real/imag split into
  two accumulating matmuls; bias folded into the ScalarE PSUM->SBUF evacuation.
- 16/32-bin bands use PE row-group packing (tile_position) so 4 small-K
  matmuls run concurrently in the 128x128 array.
"""
import numpy as np

BINS = [16] * 20 + [32] * 10 + [64] * 5 + [65]
NB = len(BINS)  # 36
E = 128
B = 8
T = 2048
TC = 512
NCHUNK = T // TC
EPS = 1e-5

# Tile layout: bins packed densely, but band 34 (64 bins) and band 35 (65
# bins) get their own tiles so no band straddles a 128-partition tile.
# tiles 0..6: bins 0..895 (128 each), tile 7: bins 896..959 (band 34),
# tile 8: bins 960..1024 (band 35).
TILE_BIN_START = [0, 128, 256, 384, 512, 640, 768, 896, 960]
TILE_BIN_COUNT = [128, 128, 128, 128, 128, 128, 128, 64, 65]
NTILES = 9

BAND_START = np.cumsum([0] + BINS)[:-1]  # bin index where each band starts


def _band_layout():
    """Per band: (tile_j, local_row0, nbins, kslot_base, kslot_size, parity)."""
    layout = []
    for beta, nb in enumerate(BINS):
        b0 = int(BAND_START[beta])
        for j in range(NTILES):
            if TILE_BIN_START[j] <= b0 < TILE_BIN_START[j] + TILE_BIN_COUNT[j]:
                break
        r0 = b0 - TILE_BIN_START[j]
        assert r0 + nb <= TILE_BIN_COUNT[j], (beta, j, r0, nb)
        if nb == 16:
            kbase = (r0 // 32) * 32
            ksize = 32
            parity = (r0 // 16) % 2
        elif nb == 32:
            assert r0 % 32 == 0
            kbase, ksize, parity = r0, 32, 0
        elif nb == 64:
            assert r0 % 64 == 0
            kbase, ksize, parity = r0, 64, 0
        else:  # 65
            assert r0 == 0
            kbase, ksize, parity = 0, 65, 0
        layout.append((j, r0, nb, kbase, ksize, parity))
    return layout


BAND_LAYOUT = _band_layout()

TILE_BANDS = [[] for _ in range(NTILES)]  # per tile: list of band indices
for _beta, (_j, *_rest) in enumerate(BAND_LAYOUT):
    TILE_BANDS[_j].append(_beta)
# Emission order: round-robin across PE row-group slots so consecutive
# self-loading matmuls target different 32-row strips and overlap.
for _j in range(NTILES):
    _bs = TILE_BANDS[_j]
    _by_slot = {}
    for _b in _bs:
        _by_slot.setdefault(BAND_LAYOUT[_b][3], []).append(_b)
    _order = []
    _slots = sorted(_by_slot)
    _i = 0
    while any(_by_slot.values()):
        _s = _slots[_i % len(_slots)]
        if _by_slot[_s]:
            _order.append(_by_slot[_s].pop(0))
        _i += 1
    TILE_BANDS[_j] = _order

OUT_GROUPS = []  # per tile: list of (beta_start, count)
for _j in range(NTILES):
    _bs = sorted(TILE_BANDS[_j])
    _gs = []
    _i = 0
    while _i < len(_bs):
        _cnt = 1
        while (_cnt < 4 and _i + _cnt < len(_bs)
               and _bs[_i + _cnt] == _bs[_i] + _cnt):
            _cnt += 1
        _gs.append((_bs[_i], _cnt))
        _i += _cnt
    OUT_GROUPS.append(_gs)

_CACHE = {}


def _build(passes=1, bench=False, ablate=None):
    """Build + schedule the Bass module once. Returns (nc, names)."""
    key = ("nc", passes, bench, ablate)
    if key in _CACHE:
        return _CACHE[key]

    from contextlib import ExitStack
    import concourse.tile as tile
    from concourse import bacc, mybir

    F32 = mybir.dt.float32
    F32R = mybir.dt.float32r
    AF = mybir.ActivationFunctionType
    ALU = mybir.AluOpType

    nc = bacc.Bacc("TRN2", target_bir_lowering=False, debug=False)

    x_d = nc.dram_tensor("x", [1025, T, 2], F32R, kind="ExternalInput").ap()
    # weights packed per (c, parity): rows = tile_j*128 + local_row
    w_d = nc.dram_tensor("w", [2, 2, NTILES * 128, E], F32R,
                         kind="ExternalInput").ap()
    ind_d = nc.dram_tensor("ind", [NTILES, 128, 2 * NB], F32R,
                           kind="ExternalInput").ap()  # [:, :, :36]=mu  [36:]=sq
    sel_d = nc.dram_tensor("sel", [NTILES, NB, 128], F32R,
                           kind="ExternalInput").ap()
    bias_d = nc.dram_tensor("bias", [E, NB], F32, kind="ExternalInput").ap()
    invd_d = nc.dram_tensor("invd", [NB, 1], F32, kind="ExternalInput").ap()
    eps_d = nc.dram_tensor("eps", [NB, 1], F32, kind="ExternalInput").ap()
    if bench:
        outx_d = nc.dram_tensor("out", [E, 1], F32, kind="ExternalOutput").ap()
        out_d = None
    else:
        out_d = nc.dram_tensor("out", [E, NB, T], F32,
                               kind="ExternalOutput").ap()

    with tile.TileContext(nc) as tc, ExitStack() as ctx:
        consts = ctx.enter_context(tc.tile_pool(name="consts", bufs=1))
        xpool = ctx.enter_context(tc.tile_pool(name="x", bufs=2))
        x2pool = ctx.enter_context(tc.tile_pool(name="x2", bufs=4))
        xspool = ctx.enter_context(tc.tile_pool(name="xs", bufs=6))
        stpool = ctx.enter_context(tc.tile_pool(name="stats", bufs=2))
        outpool = ctx.enter_context(tc.tile_pool(name="out", bufs=6))
        ps_st = ctx.enter_context(tc.tile_pool(name="ps_st", bufs=1, space="PSUM"))
        ps_sel = ctx.enter_context(tc.tile_pool(name="ps_sel", bufs=1, space="PSUM"))
        ps_z = ctx.enter_context(tc.tile_pool(name="ps_z", bufs=2, space="PSUM"))
        if bench:
            drampool = ctx.enter_context(tc.tile_pool(name="dscr", bufs=1, space="DRAM"))
            out_d = drampool.tile([E, NB, T], F32, name="out_scratch")

        # chunk-0 input loads first so the pipeline ramps while the
        # constant DMAs stream behind them in the ring
        x0_pre = []
        for j in range(NTILES):
            nb = TILE_BIN_COUNT[j]
            xt = xpool.tile([128, 2 * TC], F32R, tag=f"x{j}",
                            name=f"x_0_{j}", bufs=3 if j < 4 else 2)
            nc.sync.dma_start(
                xt[0:nb, :],
                x_d[TILE_BIN_START[j]:TILE_BIN_START[j] + nb,
                    0:TC, :].rearrange("p t c -> p (t c)"))
            x0_pre.append(xt)

        # constants
        w_s = [[consts.tile([128, E], F32R, tag=f"w{c}{p}{j}", name=f"w{c}{p}{j}")
                if not (p == 1 and j > 2) else None
                for j in range(NTILES)] for c in range(2) for p in range(2)]
        # index helper: w_s[c*2+p][j]
        for c in range(2):
            for p in range(2):
                for j in range(NTILES):
                    if p == 1 and j > 2:
                        continue  # parity-1 weights only exist for 16-bin tiles
                    nc.sync.dma_start(
                        w_s[c * 2 + p][j][:],
                        w_d[c, p, j * 128:(j + 1) * 128, :])
        ind_s = [consts.tile([128, 2 * NB], F32R, tag=f"ind{j}", name=f"ind{j}")
                 for j in range(NTILES)]
        sel_s = [consts.tile([NB, 128], F32R, tag=f"sel{j}", name=f"sel{j}")
                 for j in range(NTILES)]
        for j in range(NTILES):
            nc.sync.dma_start(ind_s[j][:], ind_d[j])
            nc.sync.dma_start(sel_s[j][:], sel_d[j])
        bias_s = consts.tile([E, NB], F32)
        nc.sync.dma_start(bias_s[:], bias_d[:])
        invd_s = consts.tile([NB, 1], F32)
        nc.sync.dma_start(invd_s[:], invd_d[:])
        eps_s = consts.tile([NB, 1], F32)
        nc.sync.dma_start(eps_s[:], eps_d[:])

        def load_x(k):
            t0 = (k % NCHUNK) * TC
            xts = []
            for j in range(NTILES):
                nb = TILE_BIN_COUNT[j]
                xt = xpool.tile([128, 2 * TC], F32R, tag=f"x{j}",
                                name=f"x_{k}_{j}", bufs=3 if j < 4 else 2)
                src = x_d[TILE_BIN_START[j]:TILE_BIN_START[j] + nb,
                          t0:t0 + TC, :].rearrange("p t c -> p (t c)")
                nc.sync.dma_start(xt[0:nb, :], src)
                xts.append(xt)
            return xts

        def stage_stats(k, xts=None):
            """Load chunk k, square, stats matmuls, produce r_sb/u_sb."""
            t0 = (k % NCHUNK) * TC
            if xts is None:
                xts = load_x(k)

            # squares: split DVE/ACT to balance engine load (first three
            # tiles on ScalarE Square, rest on VectorE)
            x2s = []
            for j in range(NTILES):
                nb = TILE_BIN_COUNT[j]
                x2 = x2pool.tile([128, 2 * TC], F32R, tag="x2", name=f"x2_{k}_{j}")
                if j < 3:
                    nc.scalar.activation(x2[0:nb, :], xts[j][0:nb, :], AF.Square)
                else:
                    nc.vector.tensor_mul(x2[0:nb, :],
                                         xts[j][0:nb, :].bitcast(F32),
                                         xts[j][0:nb, :].bitcast(F32))
                x2s.append(x2)

            # stats matmuls: mu_ps/sq_ps [36, TC] accumulated over tiles+c
            if ablate == "dma":
                return None, None, None
            mu_ps = ps_st.tile([NB, TC], F32, tag="mu", name=f"mu_{k}")
            sq_ps = ps_st.tile([NB, TC], F32, tag="sq", name=f"sq_{k}")
            n_acc = NTILES * 2
            i = 0
            for j in range(NTILES):
                nb = TILE_BIN_COUNT[j]
                for c in range(2):
                    xv = xts[j][0:nb, :].rearrange("p (t c) -> p t c", c=2)
                    x2v = x2s[j][0:nb, :].rearrange("p (t c) -> p t c", c=2)
                    nc.tensor.matmul(mu_ps[:], ind_s[j][0:nb, 0:NB],
                                     xv[:, :, c], start=(i == 0),
                                     stop=(i == n_acc - 1), skip_group_check=True)
                    nc.tensor.matmul(sq_ps[:], ind_s[j][0:nb, NB:2 * NB],
                                     x2v[:, :, c], start=(i == 0),
                                     stop=(i == n_acc - 1), skip_group_check=True)
                    i += 1

            # stats post: r = 1/sqrt(var+eps), u = mu*r  (rows [36, TC])
            mu_sb = stpool.tile([NB, TC], F32, tag="mu_sb", name=f"mu_sb_{k}", bufs=2)
            nc.vector.tensor_scalar(mu_sb[:], mu_ps[:], invd_s[:, 0:1], None,
                                    ALU.mult)
            mu2 = stpool.tile([NB, TC], F32, tag="mu2", name=f"mu2_{k}", bufs=1)
            nc.vector.tensor_mul(mu2[:], mu_sb[:], mu_sb[:])
            var = stpool.tile([NB, TC], F32, tag="var", name=f"var_{k}", bufs=1)
            # var = sq * invd - mu^2
            nc.vector.scalar_tensor_tensor(var[:], sq_ps[:], invd_s[:, 0:1],
                                           mu2[:], ALU.mult, ALU.subtract)
            sd = stpool.tile([NB, TC], F32, tag="sd", name=f"sd_{k}", bufs=1)
            nc.scalar.activation(sd[:], var[:], AF.Sqrt, bias=eps_s[:, 0:1])
            rr = stpool.tile([NB, TC], F32, tag="rr", name=f"rr_{k}", bufs=2)
            scr = stpool.tile([NB, TC], F32, tag="scr", name=f"scr_{k}", bufs=1)
            nc.vector.reciprocal_approx_accurate(rr[:], sd[:], scr[:])
            r_sb = stpool.tile([NB, TC], F32R, tag="r_sb", name=f"r_sb_{k}")
            nc.vector.tensor_copy(r_sb[:], rr[:])
            u_sb = stpool.tile([NB, TC], F32R, tag="u_sb", name=f"u_sb_{k}")
            nc.vector.tensor_mul(u_sb[:], mu_sb[:], rr[:])
            return xts, r_sb, u_sb

        def stage_z(k, xts, r_sb, u_sb):
            """Selects, normalize, per-band matmuls, evac, store for chunk k."""
            if ablate == "dma":
                return
            t0 = (k % NCHUNK) * TC
            for j in range(NTILES):
                nb = TILE_BIN_COUNT[j]
                r2d = ps_sel.tile([128, 2 * TC], F32, tag="r2d", name=f"r2d_{k}_{j}")
                m2d = ps_sel.tile([128, 2 * TC], F32, tag="m2d", name=f"m2d_{k}_{j}")
                H = TC // 2
                for h in range(2):
                    rv = r_sb[:, h * H:(h + 1) * H].to_broadcast((NB, H, 2))
                    uv = u_sb[:, h * H:(h + 1) * H].to_broadcast((NB, H, 2))
                    nc.tensor.matmul(
                        r2d[0:nb, 2 * h * H:2 * (h + 1) * H]
                        .rearrange("p (t c) -> p t c", c=2),
                        sel_s[j][:, 0:nb], rv, start=True, stop=True)
                    nc.tensor.matmul(
                        m2d[0:nb, 2 * h * H:2 * (h + 1) * H]
                        .rearrange("p (t c) -> p t c", c=2),
                        sel_s[j][:, 0:nb], uv, start=True, stop=True)
                # xs = x * r2d - m2d   (x * r_band - mu_band*r_band)
                xs = xspool.tile([128, 2 * TC], F32R, tag="xs", name=f"xs_{k}_{j}")
                nc.vector.tensor_mul(xs[0:nb, :], xts[j][0:nb, :].bitcast(F32),
                                     r2d[0:nb, :])
                nc.vector.tensor_sub(xs[0:nb, :], xs[0:nb, :].bitcast(F32),
                                     m2d[0:nb, :])
                xsr = xs[:]

                # ---- per-band matmuls + evacuation (stores grouped so one
                # DMA covers up to 4 consecutive bands)
                if ablate == "noz":
                    continue
                ygs = {}
                for (b0, cnt) in OUT_GROUPS[j]:
                    yg = outpool.tile([E, 4 * TC], F32, tag="y", bufs=4,
                                      name=f"y_{k}_{j}_{b0}")
                    for b in range(b0, b0 + cnt):
                        ygs[b] = (yg, b0, cnt)
                for beta in TILE_BANDS[j]:
                    _, r0, nbb, kbase, ksize, par = BAND_LAYOUT[beta]
                    zps = ps_z.tile([E, TC], F32, tag="z", name=f"z_{k}_{beta}")
                    kslice = slice(kbase, kbase + ksize)
                    tp = (kbase % 128, 0) if ksize <= 64 else (0, 0)
                    xsv = xsr[kslice, :].rearrange("p (t c) -> p t c", c=2)
                    for c in range(2):
                        nc.tensor.matmul(
                            zps[:], w_s[c * 2 + par][j][kslice, :],
                            xsv[:, :, c], start=(c == 0), stop=(c == 1),
                            tile_position=tp, skip_group_check=True)
                    yg, b0, cnt = ygs[beta]
                    sl = beta - b0
                    nc.scalar.activation(yg[:, sl * TC:(sl + 1) * TC], zps[:],
                                         AF.Identity,
                                         bias=bias_s[:, beta:beta + 1])
                for (b0, cnt) in OUT_GROUPS[j]:
                    yg = ygs[b0][0]
                    nc.sync.dma_start(
                        out_d[:, b0:b0 + cnt, t0:t0 + TC],
                        yg[:, 0:cnt * TC].rearrange("p (b t) -> p b t", t=TC))

        # software pipeline: chunk k+1's load/stats run while chunk k's
        # select/normalize/matmul phase executes (keeps PE dense across the
        # serial stats-post chain).
        if bench:
            dummy = consts.tile([E, 1], F32)
            nc.vector.tensor_copy(dummy[:], bias_s[:, 0:1])
            nc.sync.dma_start(outx_d[:], dummy[:])
        nk = NCHUNK * passes
        pending = stage_stats(0, xts=x0_pre)
        for k in range(nk):
            nxt = stage_stats(k + 1) if k + 1 < nk else None
            if pending is not None and pending[0] is not None:
                stage_z(k, *pending)
            pending = nxt

    nc.compile()
    _CACHE[key] = nc
    return nc


def _round_f32r(a):
    """Round fp32 array to fp32r grid (11-bit mantissa, round-to-nearest)."""
    u = a.astype(np.float32).view(np.uint32)
    u = (u + 0x800 + ((u >> 12) & 1)).astype(np.uint32) & np.uint32(0xFFFFF000)
    return u.view(np.float32)


def _host_constants(norm_w, norm_b, lin_w, lin_b):
    """Fold LN gamma/beta into the linear weights, pack to tile layout."""
    w_np = np.zeros((2, 2, NTILES * 128, E), dtype=np.float32)
    bias_np = np.zeros((E, NB), dtype=np.float32)
    ind_np = np.zeros((NTILES, 128, 2 * NB), dtype=np.float32)
    sel_np = np.zeros((NTILES, NB, 128), dtype=np.float32)
    invd_np = np.zeros((NB, 1), dtype=np.float32)

    for beta, nb in enumerate(BINS):
        j, r0, _, _, _, par = BAND_LAYOUT[beta]
        d = 2 * nb
        Wg = np.asarray(lin_w[beta], dtype=np.float64) * \
            np.asarray(norm_w[beta], dtype=np.float64)[None, :]      # [E, d]
        bp = np.asarray(lin_b[beta], dtype=np.float64) + \
            np.asarray(lin_w[beta], dtype=np.float64) @ \
            np.asarray(norm_b[beta], dtype=np.float64)               # [E]
        # column 2k+c of Wg -> w[c, par, tile_row(r0+k), :]
        for c in range(2):
            w_np[c, par, j * 128 + r0: j * 128 + r0 + nb, :] = \
                Wg[:, c::2].T.astype(np.float32)
        bias_np[:, beta] = bp.astype(np.float32)
        ind_np[j, r0:r0 + nb, beta] = 1.0
        ind_np[j, r0:r0 + nb, NB + beta] = 1.0
        sel_np[j, beta, r0:r0 + nb] = 1.0
        invd_np[beta, 0] = 1.0 / d

    w_np = _round_f32r(w_np)
    return w_np, bias_np, ind_np, sel_np, invd_np


def kernel(spec_ri, norm_w, norm_b, lin_w, lin_b):
    from concourse import bass_utils

    spec = _round_f32r(np.asarray(spec_ri, dtype=np.float32))
    assert spec.shape == (B, 1025, T, 2), spec.shape

    nc = _build()
    w_np, bias_np, ind_np, sel_np, invd_np = _host_constants(
        norm_w, norm_b, lin_w, lin_b)

    in_maps = []
    for b in range(B):
        in_maps.append({
            "x": np.ascontiguousarray(spec[b]),
            "w": w_np, "ind": ind_np, "sel": sel_np,
            "bias": bias_np, "invd": invd_np,
            "eps": np.full((NB, 1), EPS, dtype=np.float32),
        })
    res = bass_utils.run_bass_kernel_spmd(nc, in_maps, core_ids=list(range(B)))
    out = np.stack([res.results[b]["out"] for b in range(B)], axis=0)
    return out


# revision 28
# speedup vs baseline: 48.4758x; 1.0188x over previous
"""BandSplit (LayerNorm + per-band Linear) Trainium2 kernel.

Strategy:
- Data-parallel over batch: 8 batch elements -> 8 NeuronCores (SPMD, no
  collectives).
- Per core: x = spec_ri[b] with shape (1025, 2048, 2) -> out (128, 36, 2048).
- Frames processed in 4 chunks of TC=512.
- Bins packed on SBUF partitions in 9 tiles of <=128 rows; free dim is the
  contiguous (t, c) pair stream, so HBM reads are fully contiguous per bin.
- Per-frame LayerNorm stats (mean, mean-square) for all 36 bands computed with
  indicator matmuls (contract over partitions); rsqrt via sqrt + accurate
  reciprocal; per-band (r, u=mu*r) rows broadcast back to bin-partition layout
  with selector matmuls (step-0 column duplication for the (t,c) interleave);
  normalization on DVE; per-band Linear as fp32r matmuls (TF32-like, 11-bit
  mantissa) with LayerNorm gamma folded into the weights, real/imag split into
  two accumulating matmuls; bias folded into the ScalarE PSUM->SBUF evacuation.
- 16/32-bin bands use PE row-group packing (tile_position) so 4 small-K
  matmuls run concurrently in the 128x128 array.
"""
import numpy as np

BINS = [16] * 20 + [32] * 10 + [64] * 5 + [65]
NB = len(BINS)  # 36
E = 128
B = 8
T = 2048
TC = 512
NCHUNK = T // TC
EPS = 1e-5

# Tile layout: bins packed densely, but band 34 (64 bins) and band 35 (65
# bins) get their own tiles so no band straddles a 128-partition tile.
# tiles 0..6: bins 0..895 (128 each), tile 7: bins 896..959 (band 34),
# tile 8: bins 960..1024 (band 35).
TILE_BIN_START = [0, 128, 256, 384, 512, 640, 768, 896, 960]
TILE_BIN_COUNT = [128, 128, 128, 128, 128, 128, 128, 64, 65]
NTILES = 9

BAND_START = np.cumsum([0] + BINS)[:-1]  # bin index where each band starts


def _band_layout():
    """Per band: (tile_j, local_row0, nbins, kslot_base, kslot_size, parity)."""
    layout = []
    for beta, nb in enumerate(BINS):
        b0 = int(BAND_START[beta])
        for j in range(NTILES):
            if TILE_BIN_START[j] <= b0 < TILE_BIN_START[j] + TILE_BIN_COUNT[j]:
                break
        r0 = b0 - TILE_BIN_START[j]
        assert r0 + nb <= TILE_BIN_COUNT[j], (beta, j, r0, nb)
        if nb == 16:
            kbase = (r0 // 32) * 32
            ksize = 32
            parity = (r0 // 16) % 2
        elif nb == 32:
            assert r0 % 32 == 0
            kbase, ksize, parity = r0, 32, 0
        elif nb == 64:
            assert r0 % 64 == 0
            kbase, ksize, parity = r0, 64, 0
        else:  # 65
            assert r0 == 0
            kbase, ksize, parity = 0, 65, 0
        layout.append((j, r0, nb, kbase, ksize, parity))
    return layout


BAND_LAYOUT = _band_layout()

TILE_BANDS = [[] for _ in range(NTILES)]  # per tile: list of band indices
for _beta, (_j, *_rest) in enumerate(BAND_LAYOUT):
    TILE_BANDS[_j].append(_beta)
# Emission order: round-robin across PE row-group slots so consecutive
# self-loading matmuls target different 32-row strips and overlap.
for _j in range(NTILES):
    _bs = TILE_BANDS[_j]
    _by_slot = {}
    for _b in _bs:
        _by_slot.setdefault(BAND_LAYOUT[_b][3], []).append(_b)
    _order = []
    _slots = sorted(_by_slot)
    _i = 0
    while any(_by_slot.values()):
        _s = _slots[_i % len(_slots)]
        if _by_slot[_s]:
            _order.append(_by_slot[_s].pop(0))
        _i += 1
    TILE_BANDS[_j] = _order

OUT_GROUPS = []  # per tile: list of (beta_start, count)
for _j in range(NTILES):
    _bs = sorted(TILE_BANDS[_j])
    _gs = []
    _i = 0
    while _i < len(_bs):
        _cnt = 1
        while (_cnt < 4 and _i + _cnt < len(_bs)
               and _bs[_i + _cnt] == _bs[_i] + _cnt):
            _cnt += 1
        _gs.append((_bs[_i], _cnt))
        _i += _cnt
    OUT_GROUPS.append(_gs)

_CACHE = {}


def _build(passes=1, bench=False, ablate=None):
    """Build + schedule the Bass module once. Returns (nc, names)."""
    key = ("nc", passes, bench, ablate)
    if key in _CACHE:
        return _CACHE[key]

    from contextlib import ExitStack
    import concourse.tile as tile
    from concourse import bacc, mybir

    F32 = mybir.dt.float32
    F32R = mybir.dt.float32r
    AF = mybir.ActivationFunctionType
    ALU = mybir.AluOpType

    nc = bacc.Bacc("TRN2", target_bir_lowering=False, debug=False)

    x_d = nc.dram_tensor("x", [1025, T, 2], F32R, kind="ExternalInput").ap()
    # weights packed per (c, parity): rows = tile_j*128 + local_row
    w_d = nc.dram_tensor("w", [2, 2, NTILES * 128, E], F32R,
                         kind="ExternalInput").ap()
    ind_d = nc.dram_tensor("ind", [NTILES, 128, 2 * NB], F32R,
                           kind="ExternalInput").ap()  # [:, :, :36]=mu  [36:]=sq
    sel_d = nc.dram_tensor("sel", [NTILES, NB, 128], F32R,
                           kind="ExternalInput").ap()
    bias_d = nc.dram_tensor("bias", [E, NB], F32, kind="ExternalInput").ap()
    invd_d = nc.dram_tensor("invd", [NB, 1], F32, kind="ExternalInput").ap()
    eps_d = nc.dram_tensor("eps", [NB, 1], F32, kind="ExternalInput").ap()
    if bench:
        outx_d = nc.dram_tensor("out", [E, 1], F32, kind="ExternalOutput").ap()
        out_d = None
    else:
        out_d = nc.dram_tensor("out", [E, NB, T], F32,
                               kind="ExternalOutput").ap()

    with tile.TileContext(nc) as tc, ExitStack() as ctx:
        consts = ctx.enter_context(tc.tile_pool(name="consts", bufs=1))
        xpool = ctx.enter_context(tc.tile_pool(name="x", bufs=2))
        x2pool = ctx.enter_context(tc.tile_pool(name="x2", bufs=3))
        xspool = ctx.enter_context(tc.tile_pool(name="xs", bufs=4))
        stpool = ctx.enter_context(tc.tile_pool(name="stats", bufs=2))
        outpool = ctx.enter_context(tc.tile_pool(name="out", bufs=6))
        ps_st = ctx.enter_context(tc.tile_pool(name="ps_st", bufs=1, space="PSUM"))
        ps_sel = ctx.enter_context(tc.tile_pool(name="ps_sel", bufs=1, space="PSUM"))
        ps_z = ctx.enter_context(tc.tile_pool(name="ps_z", bufs=2, space="PSUM"))
        if bench:
            drampool = ctx.enter_context(tc.tile_pool(name="dscr", bufs=1, space="DRAM"))
            out_d = drampool.tile([E, NB, T], F32, name="out_scratch")

        # chunk-0 input loads first so the pipeline ramps while the
        # constant DMAs stream behind them in the ring
        x0_pre = []
        for j in range(NTILES):
            nb = TILE_BIN_COUNT[j]
            xt = xpool.tile([128, 2 * TC], F32R, tag=f"x{j}",
                            name=f"x_0_{j}", bufs=3 if j < 7 else 2)
            nc.sync.dma_start(
                xt[0:nb, :],
                x_d[TILE_BIN_START[j]:TILE_BIN_START[j] + nb,
                    0:TC, :].rearrange("p t c -> p (t c)"))
            x0_pre.append(xt)

        # constants
        w_s = [[consts.tile([128, E], F32R, tag=f"w{c}{p}{j}", name=f"w{c}{p}{j}")
                if not (p == 1 and j > 2) else None
                for j in range(NTILES)] for c in range(2) for p in range(2)]
        # index helper: w_s[c*2+p][j]
        for c in range(2):
            for p in range(2):
                for j in range(NTILES):
                    if p == 1 and j > 2:
                        continue  # parity-1 weights only exist for 16-bin tiles
                    nc.sync.dma_start(
                        w_s[c * 2 + p][j][:],
                        w_d[c, p, j * 128:(j + 1) * 128, :])
        ind_s = [consts.tile([128, 2 * NB], F32R, tag=f"ind{j}", name=f"ind{j}")
                 for j in range(NTILES)]
        sel_s = [consts.tile([NB, 128], F32R, tag=f"sel{j}", name=f"sel{j}")
                 for j in range(NTILES)]
        for j in range(NTILES):
            nc.sync.dma_start(ind_s[j][:], ind_d[j])
            nc.sync.dma_start(sel_s[j][:], sel_d[j])
        bias_s = consts.tile([E, NB], F32)
        nc.sync.dma_start(bias_s[:], bias_d[:])
        invd_s = consts.tile([NB, 1], F32)
        nc.sync.dma_start(invd_s[:], invd_d[:])
        eps_s = consts.tile([NB, 1], F32)
        nc.sync.dma_start(eps_s[:], eps_d[:])

        def load_x(k):
            t0 = (k % NCHUNK) * TC
            xts = []
            for j in range(NTILES):
                nb = TILE_BIN_COUNT[j]
                xt = xpool.tile([128, 2 * TC], F32R, tag=f"x{j}",
                                name=f"x_{k}_{j}", bufs=3 if j < 7 else 2)
                src = x_d[TILE_BIN_START[j]:TILE_BIN_START[j] + nb,
                          t0:t0 + TC, :].rearrange("p t c -> p (t c)")
                nc.sync.dma_start(xt[0:nb, :], src)
                xts.append(xt)
            return xts

        def stage_stats(k, xts=None):
            """Load chunk k, square, stats matmuls, produce r_sb/u_sb."""
            t0 = (k % NCHUNK) * TC
            if xts is None:
                xts = load_x(k)

            # squares: split DVE/ACT to balance engine load (first three
            # tiles on ScalarE Square, rest on VectorE)
            x2s = []
            for j in range(NTILES):
                nb = TILE_BIN_COUNT[j]
                x2 = x2pool.tile([128, 2 * TC], F32R, tag="x2", name=f"x2_{k}_{j}")
                if j < 3:
                    nc.scalar.activation(x2[0:nb, :], xts[j][0:nb, :], AF.Square)
                else:
                    nc.vector.tensor_mul(x2[0:nb, :],
                                         xts[j][0:nb, :].bitcast(F32),
                                         xts[j][0:nb, :].bitcast(F32))
                x2s.append(x2)

            # stats matmuls: mu_ps/sq_ps [36, TC] accumulated over tiles+c
            if ablate == "dma":
                return None, None, None
            mu_ps = ps_st.tile([NB, TC], F32, tag="mu", name=f"mu_{k}")
            sq_ps = ps_st.tile([NB, TC], F32, tag="sq", name=f"sq_{k}")
            n_acc = NTILES * 2
            i = 0
            for j in range(NTILES):
                nb = TILE_BIN_COUNT[j]
                for c in range(2):
                    xv = xts[j][0:nb, :].rearrange("p (t c) -> p t c", c=2)
                    x2v = x2s[j][0:nb, :].rearrange("p (t c) -> p t c", c=2)
                    nc.tensor.matmul(mu_ps[:], ind_s[j][0:nb, 0:NB],
                                     xv[:, :, c], start=(i == 0),
                                     stop=(i == n_acc - 1), skip_group_check=True)
                    nc.tensor.matmul(sq_ps[:], ind_s[j][0:nb, NB:2 * NB],
                                     x2v[:, :, c], start=(i == 0),
                                     stop=(i == n_acc - 1), skip_group_check=True)
                    i += 1

            # stats post: r = 1/sqrt(var+eps), u = mu*r  (rows [36, TC])
            mu_sb = stpool.tile([NB, TC], F32, tag="mu_sb", name=f"mu_sb_{k}", bufs=2)
            nc.vector.tensor_scalar(mu_sb[:], mu_ps[:], invd_s[:, 0:1], None,
                                    ALU.mult)
            mu2 = stpool.tile([NB, TC], F32, tag="mu2", name=f"mu2_{k}", bufs=1)
            nc.vector.tensor_mul(mu2[:], mu_sb[:], mu_sb[:])
            var = stpool.tile([NB, TC], F32, tag="var", name=f"var_{k}", bufs=1)
            # var = sq * invd - mu^2
            nc.vector.scalar_tensor_tensor(var[:], sq_ps[:], invd_s[:, 0:1],
                                           mu2[:], ALU.mult, ALU.subtract)
            sd = stpool.tile([NB, TC], F32, tag="sd", name=f"sd_{k}", bufs=1)
            nc.scalar.activation(sd[:], var[:], AF.Sqrt, bias=eps_s[:, 0:1])
            rr = stpool.tile([NB, TC], F32, tag="rr", name=f"rr_{k}", bufs=2)
            scr = stpool.tile([NB, TC], F32, tag="scr", name=f"scr_{k}", bufs=1)
            nc.vector.reciprocal_approx_accurate(rr[:], sd[:], scr[:])
            r_sb = stpool.tile([NB, TC], F32R, tag="r_sb", name=f"r_sb_{k}")
            nc.vector.tensor_copy(r_sb[:], rr[:])
            u_sb = stpool.tile([NB, TC], F32R, tag="u_sb", name=f"u_sb_{k}")
            nc.vector.tensor_mul(u_sb[:], mu_sb[:], rr[:])
            return xts, r_sb, u_sb

        def stage_z(k, xts, r_sb, u_sb):
            """Selects, normalize, per-band matmuls, evac, store for chunk k."""
            if ablate == "dma":
                return
            t0 = (k % NCHUNK) * TC
            for j in range(NTILES):
                nb = TILE_BIN_COUNT[j]
                # selects and normalize in two half-T pieces with 1-bank psum
                # tiles double-buffered, so select matmuls of piece h+1
                # overlap the DVE normalize of piece h
                xs = xspool.tile([128, 2 * TC], F32R, tag="xs", name=f"xs_{k}_{j}")
                H = TC // 2
                for h in range(2):
                    r2d = ps_sel.tile([128, TC], F32, tag="r2d", bufs=2,
                                      name=f"r2d_{k}_{j}_{h}")
                    m2d = ps_sel.tile([128, TC], F32, tag="m2d", bufs=2,
                                      name=f"m2d_{k}_{j}_{h}")
                    rv = r_sb[:, h * H:(h + 1) * H].to_broadcast((NB, H, 2))
                    uv = u_sb[:, h * H:(h + 1) * H].to_broadcast((NB, H, 2))
                    nc.tensor.matmul(
                        r2d[0:nb, :].rearrange("p (t c) -> p t c", c=2),
                        sel_s[j][:, 0:nb], rv, start=True, stop=True)
                    nc.tensor.matmul(
                        m2d[0:nb, :].rearrange("p (t c) -> p t c", c=2),
                        sel_s[j][:, 0:nb], uv, start=True, stop=True)
                    sl = slice(h * TC, (h + 1) * TC)
                    nc.vector.tensor_mul(xs[0:nb, sl],
                                         xts[j][0:nb, sl].bitcast(F32),
                                         r2d[0:nb, :])
                    nc.vector.tensor_sub(xs[0:nb, sl],
                                         xs[0:nb, sl].bitcast(F32),
                                         m2d[0:nb, :])
                xsr = xs[:]

                # ---- per-band matmuls + evacuation (stores grouped so one
                # DMA covers up to 4 consecutive bands)
                if ablate == "noz":
                    continue
                ygs = {}
                for (b0, cnt) in OUT_GROUPS[j]:
                    yg = outpool.tile([E, 4 * TC], F32, tag="y", bufs=4,
                                      name=f"y_{k}_{j}_{b0}")
                    for b in range(b0, b0 + cnt):
                        ygs[b] = (yg, b0, cnt)
                for beta in TILE_BANDS[j]:
                    _, r0, nbb, kbase, ksize, par = BAND_LAYOUT[beta]
                    zps = ps_z.tile([E, TC], F32, tag="z", name=f"z_{k}_{beta}")
                    kslice = slice(kbase, kbase + ksize)
                    tp = (kbase % 128, 0) if ksize <= 64 else (0, 0)
                    xsv = xsr[kslice, :].rearrange("p (t c) -> p t c", c=2)
                    for c in range(2):
                        nc.tensor.matmul(
                            zps[:], w_s[c * 2 + par][j][kslice, :],
                            xsv[:, :, c], start=(c == 0), stop=(c == 1),
                            tile_position=tp, skip_group_check=True)
                    yg, b0, cnt = ygs[beta]
                    sl = beta - b0
                    nc.scalar.activation(yg[:, sl * TC:(sl + 1) * TC], zps[:],
                                         AF.Identity,
                                         bias=bias_s[:, beta:beta + 1])
                for (b0, cnt) in OUT_GROUPS[j]:
                    yg = ygs[b0][0]
                    nc.sync.dma_start(
                        out_d[:, b0:b0 + cnt, t0:t0 + TC],
                        yg[:, 0:cnt * TC].rearrange("p (b t) -> p b t", t=TC))

        # software pipeline: chunk k+1's load/stats run while chunk k's
        # select/normalize/matmul phase executes (keeps PE dense across the
        # serial stats-post chain).
        if bench:
            dummy = consts.tile([E, 1], F32)
            nc.vector.tensor_copy(dummy[:], bias_s[:, 0:1])
            nc.sync.dma_start(outx_d[:], dummy[:])
        nk = NCHUNK * passes
        pending = stage_stats(0, xts=x0_pre)
        for k in range(nk):
            nxt = stage_stats(k + 1) if k + 1 < nk else None
            if pending is not None and pending[0] is not None:
                stage_z(k, *pending)
            pending = nxt

    nc.compile()
    _CACHE[key] = nc
    return nc


def _round_f32r(a):
    """Round fp32 array to fp32r grid (11-bit mantissa, round-to-nearest)."""
    u = a.astype(np.float32).view(np.uint32)
    u = (u + 0x800 + ((u >> 12) & 1)).astype(np.uint32) & np.uint32(0xFFFFF000)
    return u.view(np.float32)


def _host_constants(norm_w, norm_b, lin_w, lin_b):
    """Fold LN gamma/beta into the linear weights, pack to tile layout."""
    w_np = np.zeros((2, 2, NTILES * 128, E), dtype=np.float32)
    bias_np = np.zeros((E, NB), dtype=np.float32)
    ind_np = np.zeros((NTILES, 128, 2 * NB), dtype=np.float32)
    sel_np = np.zeros((NTILES, NB, 128), dtype=np.float32)
    invd_np = np.zeros((NB, 1), dtype=np.float32)

    for beta, nb in enumerate(BINS):
        j, r0, _, _, _, par = BAND_LAYOUT[beta]
        d = 2 * nb
        Wg = np.asarray(lin_w[beta], dtype=np.float64) * \
            np.asarray(norm_w[beta], dtype=np.float64)[None, :]      # [E, d]
        bp = np.asarray(lin_b[beta], dtype=np.float64) + \
            np.asarray(lin_w[beta], dtype=np.float64) @ \
            np.asarray(norm_b[beta], dtype=np.float64)               # [E]
        # column 2k+c of Wg -> w[c, par, tile_row(r0+k), :]
        for c in range(2):
            w_np[c, par, j * 128 + r0: j * 128 + r0 + nb, :] = \
                Wg[:, c::2].T.astype(np.float32)
        bias_np[:, beta] = bp.astype(np.float32)
        ind_np[j, r0:r0 + nb, beta] = 1.0
        ind_np[j, r0:r0 + nb, NB + beta] = 1.0
        sel_np[j, beta, r0:r0 + nb] = 1.0
        invd_np[beta, 0] = 1.0 / d

    w_np = _round_f32r(w_np)
    return w_np, bias_np, ind_np, sel_np, invd_np


def kernel(spec_ri, norm_w, norm_b, lin_w, lin_b):
    from concourse import bass_utils

    spec = _round_f32r(np.asarray(spec_ri, dtype=np.float32))
    assert spec.shape == (B, 1025, T, 2), spec.shape

    nc = _build()
    w_np, bias_np, ind_np, sel_np, invd_np = _host_constants(
        norm_w, norm_b, lin_w, lin_b)

    in_maps = []
    for b in range(B):
        in_maps.append({
            "x": np.ascontiguousarray(spec[b]),
            "w": w_np, "ind": ind_np, "sel": sel_np,
            "bias": bias_np, "invd": invd_np,
            "eps": np.full((NB, 1), EPS, dtype=np.float32),
        })
    res = bass_utils.run_bass_kernel_spmd(nc, in_maps, core_ids=list(range(B)))
    out = np.stack([res.results[b]["out"] for b in range(B)], axis=0)
    return out
